# revision 6
# baseline (speedup 1.0000x reference)
"""Trainium2 Bass kernel for nn_Explicit_corr (template/candidate cosine
correlation with unfold): 8-core pure data parallel over the batch dim.

Inputs (full): template_feature (64,256,4,4), candidate_feature (64,256,20,20)
Outputs (full): common_template (64,257,4,4), common_search (64,257,4,4,17,17),
                weight_norm (64,1,4,4), wc_norm (64,1,4,4,17,17)
"""

import numpy as np

from concourse import bacc, mybir, tile
from concourse.bass_utils import run_bass_kernel_spmd

F32 = mybir.dt.float32
AX = mybir.AxisListType.X
EXP = mybir.ActivationFunctionType.Exp

B_TOTAL = 64
N_CORES = 8
B = B_TOTAL // N_CORES  # samples per core
C = 256
S = 4      # template spatial / unfold kernel size
W = 20     # candidate spatial
P = 17     # sliding positions (20 - 4 + 1)
PP = P * P         # 289
SS = S * S         # 16
NCH = C // 128     # channel chunks of 128


def _out16_view(ap4):
    """(4,4,17,17) DRAM output view as contiguous (16,289)."""
    return ap4.rearrange("x y p q -> (x y p q)").rearrange("(a b) -> a b", a=SS)


def build_nc(reps: int = 1):
    nc = bacc.Bacc()
    tf = nc.dram_tensor("tf", [B, C, S, S], F32, kind="ExternalInput")
    cf = nc.dram_tensor("cf", [B, C, W, W], F32, kind="ExternalInput")
    ct = nc.dram_tensor("ct", [B, C + 1, S, S], F32, kind="ExternalOutput")
    cs = nc.dram_tensor("cs", [B, C + 1, S, S, P, P], F32, kind="ExternalOutput")
    wn = nc.dram_tensor("wn", [B, 1, S, S], F32, kind="ExternalOutput")
    wcn = nc.dram_tensor("wcn", [B, 1, S, S, P, P], F32, kind="ExternalOutput")

    with tile.TileContext(nc) as tc:
        with (
            tc.tile_pool(name="const", bufs=1) as cpool,
            tc.tile_pool(name="io", bufs=3) as iop,
            tc.tile_pool(name="work", bufs=2) as wp,
            tc.tile_pool(name="prod", bufs=4) as prodp,
            tc.tile_pool(name="psum", bufs=2, space="PSUM") as psp,
            tc.tile_pool(name="psum1", bufs=1, space="PSUM") as psp1,
            tc.tile_pool(name="dram", bufs=2, space="DRAM") as dramp,
        ):
            # constants
            e_all = cpool.tile([128, SS, SS], F32)       # e_all[:, ij, :] = one-hot col ij
            nc.vector.memset(e_all[:], 0.0)
            for ij in range(SS):
                nc.vector.memset(e_all[:, ij, ij : ij + 1], 1.0)
            ones_c16 = cpool.tile([128, SS], F32)        # all ones: U broadcast
            nc.vector.memset(ones_c16[:], 1.0)
            ones_c1 = cpool.tile([128, 1], F32)          # template reduces
            nc.vector.memset(ones_c1[:], 1.0)
            ones_16 = cpool.tile([SS, SS], F32)          # partition-sum of [16,1]
            nc.vector.memset(ones_16[:], 1.0)

            for _ in range(reps):
                for b in range(B):
                    # ---------------- template branch ----------------
                    tt = []
                    for k in range(NCH):
                        t_k = iop.tile([128, SS], F32, tag="t")
                        nc.sync.dma_start(
                            t_k[:],
                            tf[b, k * 128 : (k + 1) * 128].rearrange("c x y -> c (x y)"),
                        )
                        tt.append(t_k)
                    psT = psp1.tile([1, 2 * SS + 1], F32, tag="psT")
                    for k in range(NCH):
                        tsum = wp.tile([128, 1], F32, tag="tsum")
                        nc.vector.reduce_sum(tsum[:], tt[k][:], axis=AX)
                        tcat = wp.tile([128, 2 * SS + 1], F32, tag="tcat")
                        nc.vector.tensor_scalar_mul(tcat[:, 0:SS], tt[k][:], tsum[:])
                        nc.scalar.square(tcat[:, SS : 2 * SS], tt[k][:])
                        nc.scalar.square(tcat[:, 2 * SS : 2 * SS + 1], tsum[:])
                        nc.tensor.matmul(
                            psT[:], ones_c1[:], tcat[:], start=(k == 0), stop=(k == NCH - 1)
                        )
                    ut = wp.tile([1, 1], F32, tag="ut")
                    nc.scalar.copy(out=ut[:], in_=psT[:, 2 * SS : 2 * SS + 1])
                    tden = wp.tile([1, SS], F32, tag="tden")
                    nc.vector.tensor_scalar_mul(tden[:], psT[:, SS : 2 * SS], ut[:])
                    nc.scalar.sqrt(tden[:], tden[:])
                    trd = wp.tile([1, SS], F32, tag="trd")
                    nc.vector.reciprocal(trd[:], tden[:])
                    tw = wp.tile([1, SS], F32, tag="tw")
                    nc.vector.tensor_mul(tw[:], psT[:, 0:SS], trd[:])
                    te = wp.tile([1, SS], F32, tag="te")
                    tes = wp.tile([1, 1], F32, tag="tes")
                    nc.scalar.activation(te[:], tw[:], EXP, accum_out=tes[:])
                    trs = wp.tile([1, 1], F32, tag="trs")
                    nc.vector.reciprocal(trs[:], tes[:])
                    twn = wp.tile([1, SS], F32, tag="twn")
                    nc.vector.tensor_scalar_mul(twn[:], te[:], trs[:])
                    nc.sync.dma_start(wn[b].rearrange("o x y -> o (x y)"), twn[:])
                    nc.sync.dma_start(ct[b, C : C + 1].rearrange("o x y -> o (x y)"), twn[:])
                    for k in range(NCH):
                        nc.sync.dma_start(
                            ct[b, k * 128 : (k + 1) * 128].rearrange("c x y -> c (x y)"),
                            tt[k][:],
                        )

                    # ---------------- candidate branch ----------------
                    cc, mm = [], []
                    psU = psp.tile([SS, PP], F32, tag="psU")
                    psV = psp.tile([SS, PP], F32, tag="psV")
                    psD = psp.tile([SS, PP], F32, tag="psD")
                    for k in range(NCH):
                        c_k = iop.tile([128, W, W], F32, tag="c")
                        nc.sync.dma_start(c_k[:], cf[b, k * 128 : (k + 1) * 128])
                        cc.append(c_k)
                        # 4x4 box sum (16x the patch mean; scale cancels in cosine)
                        a1 = wp.tile([128, W, P], F32, tag="a1")
                        nc.vector.tensor_add(a1[:], c_k[:, :, 0:P], c_k[:, :, 1 : P + 1])
                        a2 = wp.tile([128, W, P], F32, tag="a2")
                        nc.vector.tensor_add(a2[:], c_k[:, :, 2 : P + 2], c_k[:, :, 3 : P + 3])
                        nc.vector.tensor_add(a1[:], a1[:], a2[:])
                        m_k = wp.tile([128, P, P], F32, tag="m")
                        nc.vector.tensor_add(m_k[:], a1[:, 0:P, :], a1[:, 1 : P + 1, :])
                        b2 = wp.tile([128, P, P], F32, tag="b2")
                        nc.vector.tensor_add(b2[:], a1[:, 2 : P + 2, :], a1[:, 3 : P + 3, :])
                        nc.vector.tensor_add(m_k[:], m_k[:], b2[:])
                        mm.append(m_k)
                        msq = wp.tile([128, P, P], F32, tag="msq")
                        nc.scalar.square(msq[:], m_k[:])
                        nc.tensor.matmul(
                            psU[:], ones_c16[:], msq[:].rearrange("c p q -> c (p q)"),
                            start=(k == 0), stop=(k == NCH - 1),
                        )
                        csq = wp.tile([128, W, W], F32, tag="csq")
                        nc.scalar.square(csq[:], c_k[:])
                        for ij in range(SS):
                            i, j = divmod(ij, S)
                            nc.tensor.matmul(
                                psV[:].rearrange("s (p q) -> s p q", p=P),
                                e_all[:, ij, :],
                                csq[:, i : i + P, j : j + P],
                                start=(k == 0 and ij == 0),
                                stop=(k == NCH - 1 and ij == SS - 1),
                            )
                    for k in range(NCH):
                        for ij in range(SS):
                            i, j = divmod(ij, S)
                            prod = prodp.tile([128, P, P], F32, tag="prod")
                            nc.vector.tensor_mul(
                                prod[:], mm[k][:], cc[k][:, i : i + P, j : j + P]
                            )
                            nc.tensor.matmul(
                                psD[:],
                                e_all[:, ij, :],
                                prod[:].rearrange("c p q -> c (p q)"),
                                start=(k == 0 and ij == 0),
                                stop=(k == NCH - 1 and ij == SS - 1),
                            )
                    u_sb = wp.tile([SS, PP], F32, tag="u_sb")
                    nc.scalar.copy(out=u_sb[:], in_=psU[:])
                    den = wp.tile([SS, PP], F32, tag="den")
                    nc.vector.tensor_mul(den[:], psV[:], u_sb[:])
                    nc.scalar.sqrt(den[:], den[:])
                    rden = wp.tile([SS, PP], F32, tag="rden")
                    nc.vector.reciprocal(rden[:], den[:])
                    wv = wp.tile([SS, PP], F32, tag="wv")
                    nc.vector.tensor_mul(wv[:], psD[:], rden[:])
                    ev = wp.tile([SS, PP], F32, tag="ev")
                    rs = wp.tile([SS, 1], F32, tag="rs")
                    nc.scalar.activation(ev[:], wv[:], EXP, accum_out=rs[:])
                    psS = psp1.tile([SS, 1], F32, tag="psS")
                    nc.tensor.matmul(psS[:], ones_16[:], rs[:], start=True, stop=True)
                    rtot = wp.tile([SS, 1], F32, tag="rtot")
                    nc.vector.reciprocal(rtot[:], psS[:])
                    sv = wp.tile([SS, PP], F32, tag="sv")
                    nc.vector.tensor_scalar_mul(sv[:], ev[:], rtot[:])
                    # The reference's raw .view() reinterprets the flat
                    # softmax vector (ij-major) as (pq, xy) then permutes:
                    # out[xy, pq] = s_flat[pq*16 + xy]. That index map is not
                    # affine in the SBUF (ij, pq) layout, so bounce the 18KB
                    # block through DRAM flat and re-read it strided.
                    sflat = dramp.tile([SS, PP], F32, tag="sflat")
                    nc.sync.dma_start(sflat[:], sv[:])
                    tv = (
                        sflat[:]
                        .rearrange("a b -> (a b)")
                        .rearrange("(a b) -> a b", b=SS)
                        .transpose([1, 0])
                    )
                    nc.sync.dma_start(_out16_view(wcn[b, 0]), tv)
                    nc.sync.dma_start(_out16_view(cs[b, C]), tv)
                    # the unfold: stream shifted windows straight out of SBUF
                    for k in range(NCH):
                        for ij in range(SS):
                            i, j = divmod(ij, S)
                            nc.sync.dma_start(
                                cs[b, k * 128 : (k + 1) * 128, i, j],
                                cc[k][:, i : i + P, j : j + P],
                            )
    nc.finalize()
    return nc


_NC_CACHE = {}


def _get_nc(reps: int = 1):
    if reps not in _NC_CACHE:
        _NC_CACHE[reps] = build_nc(reps)
    return _NC_CACHE[reps]


def kernel(template_feature, candidate_feature, reps: int = 1):
    tf = np.ascontiguousarray(np.asarray(template_feature, dtype=np.float32))
    cf = np.ascontiguousarray(np.asarray(candidate_feature, dtype=np.float32))
    nc = _get_nc(reps)
    in_maps = [
        {"tf": tf[i * B : (i + 1) * B], "cf": cf[i * B : (i + 1) * B]}
        for i in range(N_CORES)
    ]
    res = run_bass_kernel_spmd(nc, in_maps, core_ids=list(range(N_CORES))).results
    ct = np.concatenate([r["ct"] for r in res], axis=0)
    cs = np.concatenate([r["cs"] for r in res], axis=0)
    wn = np.concatenate([r["wn"] for r in res], axis=0)
    wcn = np.concatenate([r["wcn"] for r in res], axis=0)
    return ct, cs, wn, wcn


# revision 31
# speedup vs baseline: 457.3069x; 457.3069x over previous
"""Trainium2 Bass kernel for nn_Explicit_corr (template/candidate cosine
correlation with unfold): 8-core pure data parallel over the batch dim.

Inputs (full): template_feature (64,256,4,4), candidate_feature (64,256,20,20)
Outputs (full): common_template (64,257,4,4), common_search (64,257,4,4,17,17),
                weight_norm (64,1,4,4), wc_norm (64,1,4,4,17,17)

Per-core structure (8 samples):
- candidate loaded once to SBUF [128c, 20, 20] x2 chunks
- the unfold output (the memory-bound bulk: 38MB/core) is materialized in
  SBUF as [128c, 16shift, 289pos] (contiguous per channel) so the store is
  one ~2.4MB line-rate DMA per (sample, chunk)
- channel reductions (cosine dots, norms) via TensorE matmuls with one-hot
  column lhsT into PSUM [16shift, 289pos]
- candidate-window sq-norms V from a single channel-sum [16,400] + per-row
  window copies (cheaper than 32 matmuls)
- softmax epilogue on [16,289]; the reference's raw-.view() permutation is
  a flat reinterpretation (16,289)->(289,16)->T, done via SBUF->SBUF DMA
  reshape + PE transposes (all descriptors >= 64B)
"""

import dataclasses

import numpy as np

from concourse import bacc, mybir, tile
from concourse.bass_utils import run_bass_kernel_spmd

F32 = mybir.dt.float32
F32R = mybir.dt.float32r
AX = mybir.AxisListType.X
EXP = mybir.ActivationFunctionType.Exp

B_TOTAL = 64
N_CORES = 8
B = B_TOTAL // N_CORES  # samples per core
C = 256
S = 4      # template spatial / unfold kernel size
W = 20     # candidate spatial
P = 17     # sliding positions (20 - 4 + 1)
PP = P * P         # 289
SS = S * S         # 16
NCH = C // 128     # channel chunks of 128


def _win4(c_k, i):
    """[128, 4, 17, 17] overlapping-window view of c_k = [128, 20, 20]:
    dims (j:4 step1, p:17 step20, q:17 step1) — 4 unfold windows in one AP."""
    base = c_k[:, i : i + P, 0:P]
    return dataclasses.replace(base, ap=[base.ap[0], [1, S], base.ap[1], base.ap[2]])


def _out16_view(ap4):
    """(4,4,17,17) DRAM output view as contiguous (16,289)."""
    return ap4.rearrange("x y p q -> (x y p q)").rearrange("(a b) -> a b", a=SS)


def build_nc(reps: int = 1, loop: bool = False):
    nc = bacc.Bacc()
    tf = nc.dram_tensor("tf", [B, C, S, S], F32, kind="ExternalInput")
    cf = nc.dram_tensor("cf", [B, C, W, W], F32, kind="ExternalInput")
    idin = nc.dram_tensor("ident", [128, 128], F32, kind="ExternalInput")
    eain = nc.dram_tensor("eall", [128, SS, SS], F32, kind="ExternalInput")
    ct = nc.dram_tensor("ct", [B, C + 1, S, S], F32, kind="ExternalOutput")
    cs = nc.dram_tensor("cs", [B, C + 1, S, S, P, P], F32, kind="ExternalOutput")
    wn = nc.dram_tensor("wn", [B, 1, S, S], F32, kind="ExternalOutput")
    wcn = nc.dram_tensor("wcn", [B, 1, S, S, P, P], F32, kind="ExternalOutput")

    with tile.TileContext(nc) as tc:
        with (
            tc.tile_pool(name="const", bufs=1) as cpool,
            tc.tile_pool(name="io", bufs=4) as iop,
            tc.tile_pool(name="unf", bufs=3) as unfp,
            tc.tile_pool(name="work", bufs=3) as wp,
            tc.tile_pool(name="prod", bufs=6) as prodp,
            tc.tile_pool(name="psd", bufs=2, space="PSUM") as psd_p,
            tc.tile_pool(name="psv", bufs=2, space="PSUM") as psv_p,
            tc.tile_pool(name="psu", bufs=1, space="PSUM") as psu_p,
            tc.tile_pool(name="psum1", bufs=1, space="PSUM") as psp1,
        ):
            # constants
            e_sb = cpool.tile([128, SS, SS], F32)
            nc.sync.dma_start(e_sb[:], eain[:])
            e_all = cpool.tile([128, SS, SS], F32R)      # e_all[:, ij, :] = one-hot col ij
            nc.vector.tensor_copy(e_all[:], e_sb[:])     # rounded-to-f32r producer
            ones_c16 = cpool.tile([128, SS], F32)        # all ones: U broadcast
            nc.vector.memset(ones_c16[:], 1.0)
            ones_c1 = cpool.tile([128, 1], F32)          # template reduces
            nc.vector.memset(ones_c1[:], 1.0)
            ones_16 = cpool.tile([SS, SS], F32)          # partition-sum of [16,1]
            nc.vector.memset(ones_16[:], 1.0)
            ident = cpool.tile([128, 128], F32)          # PE transpose identity
            nc.sync.dma_start(ident[:], idin[:])
            # fixed product tiles, padded to even N for the fp32r matmul ISA
            # restriction; pad column initialized once and never re-written
            prodfix = [
                cpool.tile([128, PP + 1], F32R, tag=f"pf{i}", name=f"pf{i}")
                for i in range(8)
            ]
            for t in prodfix:
                nc.vector.tensor_copy(t[:, PP : PP + 1], ones_c1[:])

            def one_rep():
                for b in range(B):
                    # ---------------- template branch ----------------
                    tcb = iop.tile([128, NCH, SS], F32, tag="t")
                    nc.sync.dma_start(
                        tcb[:],
                        tf[b].rearrange("(k c) x y -> c k (x y)", k=NCH),
                    )
                    tt = [tcb[:, k, :] for k in range(NCH)]
                    psT = psp1.tile([1, 2 * SS + 1], F32, tag="psT")
                    for k in range(NCH):
                        tsum = wp.tile([128, 1], F32, tag="tsum")
                        nc.vector.reduce_sum(tsum[:], tt[k], axis=AX)
                        tcat = wp.tile([128, 2 * SS + 1], F32, tag="tcat")
                        nc.vector.tensor_scalar_mul(tcat[:, 0:SS], tt[k], tsum[:])
                        nc.vector.tensor_mul(tcat[:, SS : 2 * SS], tt[k], tt[k])
                        nc.vector.tensor_mul(tcat[:, 2 * SS : 2 * SS + 1], tsum[:], tsum[:])
                        nc.tensor.matmul(
                            psT[:], ones_c1[:], tcat[:], start=(k == 0), stop=(k == NCH - 1)
                        )
                    ut = wp.tile([1, 1], F32, tag="ut")
                    nc.scalar.copy(out=ut[:], in_=psT[:, 2 * SS : 2 * SS + 1])
                    tden = wp.tile([1, SS], F32, tag="tden")
                    nc.vector.tensor_scalar_mul(tden[:], psT[:, SS : 2 * SS], ut[:])
                    nc.scalar.sqrt(tden[:], tden[:])
                    trd = wp.tile([1, SS], F32, tag="trd")
                    nc.vector.reciprocal(trd[:], tden[:])
                    tw = wp.tile([1, SS], F32, tag="tw")
                    nc.vector.tensor_mul(tw[:], psT[:, 0:SS], trd[:])
                    te = wp.tile([1, SS], F32, tag="te")
                    tes = wp.tile([1, 1], F32, tag="tes")
                    nc.scalar.activation(te[:], tw[:], EXP, accum_out=tes[:])
                    trs = wp.tile([1, 1], F32, tag="trs")
                    nc.vector.reciprocal(trs[:], tes[:])
                    twn = wp.tile([1, SS], F32, tag="twn")
                    nc.vector.tensor_scalar_mul(twn[:], te[:], trs[:])
                    nc.sync.dma_start(wn[b].rearrange("o x y -> o (x y)"), twn[:])
                    nc.sync.dma_start(ct[b, C : C + 1].rearrange("o x y -> o (x y)"), twn[:])
                    nc.scalar.dma_start(
                        ct[b, 0:C].rearrange("(k c) x y -> c k (x y)", k=NCH), tcb[:]
                    )

                    # ---------------- candidate branch ----------------
                    psU = psu_p.tile([SS, PP], F32, tag="psU")
                    psV = psv_p.tile([SS, P, P + 1], F32, tag="psV")
                    psDs = [psd_p.tile([SS, PP + 1], F32, tag="psD", name=f"psD{kk}") for kk in range(NCH)]
                    unfs, mms = [], []
                    cb = iop.tile([128, NCH, W * W], F32, tag="c")
                    nc.scalar.dma_start(
                        cb[:], cf[b].rearrange("(k c) h w -> c k (h w)", k=NCH)
                    )
                    for k in range(NCH):
                        c_k = cb[:, k, :].rearrange("c (h w) -> c h w", h=W)
                        # materialize the unfold contiguously: 4 windows per
                        # op (strided overlapping src view), engines mixed
                        unf_k = unfp.tile([128, SS, PP], F32, tag="unf")
                        unfs.append(unf_k)
                        for i in range(S):
                            dst4 = unf_k[:, i * S : (i + 1) * S, :].rearrange(
                                "c j (p q) -> c j p q", p=P
                            )
                            win4 = _win4(c_k, i)
                            if i in (1, 3):
                                nc.vector.tensor_copy(dst4, win4)
                            elif i == 2:
                                nc.scalar.copy(out=dst4, in_=win4)
                            else:
                                nc.gpsimd.tensor_copy(dst4, win4)
                        # 4x4 box sum (16x the patch mean; scale cancels in cosine)
                        a1 = wp.tile([128, W, P], F32, tag="a1")
                        nc.vector.tensor_add(a1[:], c_k[:, :, 0:P], c_k[:, :, 1 : P + 1])
                        a2 = wp.tile([128, W, P], F32, tag="a2")
                        nc.vector.tensor_add(a2[:], c_k[:, :, 2 : P + 2], c_k[:, :, 3 : P + 3])
                        nc.vector.tensor_add(a1[:], a1[:], a2[:])
                        m_k = wp.tile([128, P, P], F32, tag="m")
                        nc.vector.tensor_add(m_k[:], a1[:, 0:P, :], a1[:, 1 : P + 1, :])
                        b2 = wp.tile([128, P, P], F32, tag="b2")
                        nc.vector.tensor_add(b2[:], a1[:, 2 : P + 2, :], a1[:, 3 : P + 3, :])
                        nc.vector.tensor_add(m_k[:], m_k[:], b2[:])
                        mms.append(m_k)
                        msq = wp.tile([128, P, P], F32, tag="msq")
                        nc.scalar.square(msq[:], m_k[:])
                        nc.tensor.matmul(
                            psU[:], ones_c16[:],
                            msq[:].rearrange("c p q -> c (p q)"),
                            start=(k == 0), stop=(k == NCH - 1),
                        )
                        csq = wp.tile([128, W, W + 1], F32R, tag="csq")
                        nc.scalar.square(csq[:, :, 0:W], c_k[:])
                        # initialize the pad column (read by the matmul when
                        # j == S-1, values land in psV's unused pad column)
                        nc.vector.tensor_copy(csq[:, :, W : W + 1], c_k[:, :, 0:1])
                        for ij in range(SS):
                            i, j = divmod(ij, S)
                            nc.tensor.matmul(
                                psV[:],
                                e_all[:, ij, :],
                                csq[:, i : i + P, j : j + P + 1],
                                start=(k == 0 and ij == 0),
                                stop=(k == NCH - 1 and ij == SS - 1),
                            )
                        # big unfold store: 128 descriptors x 18.5KB, line rate
                        # (alternate between the two HWDGE rings)
                        (nc.sync if k == 0 else nc.scalar).dma_start(
                            cs[b, k * 128 : (k + 1) * 128, 0:S].rearrange(
                                "c x y p q -> c (x y p q)"
                            ),
                            unf_k[:].rearrange("c s p -> c (s p)"),
                        )
                    # dots: lhsT = one-hot column ij; two independent
                    # accumulation chains (one per channel chunk / PSUM bank)
                    for k in range(NCH):
                        for ij in range(SS):
                            prod = prodfix[(k * SS + ij) % len(prodfix)]
                            eng = nc.vector if k == 0 else nc.gpsimd
                            eng.tensor_mul(
                                prod[:, 0:PP], mms[k][:].rearrange("c p q -> c (p q)"),
                                unfs[k][:, ij, :],
                            )
                            nc.tensor.matmul(
                                psDs[k][:], e_all[:, ij, :], prod[:],
                                start=(ij == 0), stop=(ij == SS - 1),
                            )
                    u_sb = wp.tile([SS, PP], F32, tag="u_sb")
                    nc.scalar.copy(out=u_sb[:], in_=psU[:])
                    den = wp.tile([SS, PP], F32, tag="den")
                    nc.vector.tensor_mul(
                        den[:].rearrange("s (p q) -> s p q", p=P),
                        psV[:, :, 0:P],
                        u_sb[:].rearrange("s (p q) -> s p q", p=P),
                    )
                    nc.scalar.sqrt(den[:], den[:])
                    rden = wp.tile([SS, PP], F32, tag="rden")
                    nc.vector.reciprocal(rden[:], den[:])
                    d0_sb = wp.tile([SS, PP], F32, tag="d0_sb")
                    nc.scalar.copy(out=d0_sb[:], in_=psDs[0][:, 0:PP])
                    dsum = wp.tile([SS, PP], F32, tag="dsum")
                    nc.vector.tensor_add(dsum[:], d0_sb[:], psDs[1][:, 0:PP])
                    wv = wp.tile([SS, PP], F32, tag="wv")
                    nc.vector.tensor_mul(wv[:], dsum[:], rden[:])
                    ev = wp.tile([SS, PP], F32, tag="ev")
                    rs = wp.tile([SS, 1], F32, tag="rs")
                    nc.scalar.activation(ev[:], wv[:], EXP, accum_out=rs[:])
                    psS = psp1.tile([SS, 1], F32, tag="psS")
                    nc.tensor.matmul(psS[:], ones_16[:], rs[:], start=True, stop=True)
                    rtot = wp.tile([SS, 1], F32, tag="rtot")
                    nc.vector.reciprocal(rtot[:], psS[:])
                    sv = wp.tile([SS, PP], F32, tag="sv")
                    nc.vector.tensor_scalar_mul(sv[:], ev[:], rtot[:])

                    # The reference's raw .view() permutation: flat reinterpret
                    # (16,289) -> (289,16) -> transpose. Flatten to one SBUF
                    # partition, re-split into [<=128, 16] row tiles (64B
                    # descriptors, SBUF->SBUF), PE-transpose back to [16, *].
                    sfb = wp.tile([1, SS, PP], F32, tag="sfb")
                    nc.sync.dma_start(sfb[0:1, :, :], sv[:])
                    svT = wp.tile([SS, PP], F32, tag="svT")
                    sfv = (
                        sfb[0:1, :, :]
                        .rearrange("o a b -> o (a b)")
                        .rearrange("o (r c) -> o r c", c=SS)
                    )  # (1, 289, 16) on one partition
                    for lo, hi in ((0, 128), (128, 256), (256, PP)):
                        p = hi - lo
                        rt = wp.tile([128, SS], F32, tag="rt")
                        nc.sync.dma_start(rt[:p, :], sfv[:, lo:hi, :])
                        pst = psp1.tile([SS, 128], F32, tag="pst")
                        nc.tensor.transpose(pst[:, :p], rt[:p, :], ident[:p, :p])
                        nc.scalar.copy(out=svT[:, lo:hi], in_=pst[:, :p])
                    nc.sync.dma_start(_out16_view(wcn[b, 0]), svT[:])
                    nc.sync.dma_start(_out16_view(cs[b, C]), svT[:])
            if loop and reps > 1:
                hints = (mybir.EngineType.PE, mybir.EngineType.SP, mybir.EngineType.DVE,
                         mybir.EngineType.Activation, mybir.EngineType.Pool)
                with tc.For_i(0, reps, 1, hint_engines=hints):
                    one_rep()
            else:
                for _ in range(reps):
                    one_rep()
    nc.finalize()
    return nc


_NC_CACHE = {}


def _get_nc(reps: int = 1, loop: bool = False):
    key = (reps, loop)
    if key not in _NC_CACHE:
        _NC_CACHE[key] = build_nc(reps, loop)
    return _NC_CACHE[key]


def kernel(template_feature, candidate_feature, reps: int = 1, loop: bool = False):
    tf = np.ascontiguousarray(np.asarray(template_feature, dtype=np.float32))
    cf = np.ascontiguousarray(np.asarray(candidate_feature, dtype=np.float32))
    nc = _get_nc(reps, loop)
    ident = np.eye(128, dtype=np.float32)
    eall = np.zeros((128, SS, SS), np.float32)
    for ij in range(SS):
        eall[:, ij, ij] = 1.0
    in_maps = [
        {
            "tf": tf[i * B : (i + 1) * B],
            "cf": cf[i * B : (i + 1) * B],
            "ident": ident,
            "eall": eall,
        }
        for i in range(N_CORES)
    ]
    res = run_bass_kernel_spmd(nc, in_maps, core_ids=list(range(N_CORES))).results
    ct = np.concatenate([r["ct"] for r in res], axis=0)
    cs = np.concatenate([r["cs"] for r in res], axis=0)
    wn = np.concatenate([r["wn"] for r in res], axis=0)
    wcn = np.concatenate([r["wcn"] for r in res], axis=0)
    return ct, cs, wn, wcn

